# revision 19
# baseline (speedup 1.0000x reference)
"""TRN2 Bass kernel: relu + per-row top-32 masking for x [4096, 32768] f32.

kernel(x) -> (relu(x), topk_masked) matching:
    y = relu(x); vals, idx = top_k(y, 32); xz = zeros.at[rows, idx].set(vals)

Sharding: pure data parallel over rows, 8 NeuronCores x [512, 32768].

Per-core algorithm (exact for any input with >=32 positive entries per row):
  stream x in column sub-tiles: relu on ScalarE -> write y via the ACT HWDGE
  ring (SP ring stays a pure load stream); chunk maxes on VectorE. Top-32
  chunks via 4 rounds of DVE max8 + match_replace on a copy (selection mask
  = work != orig, reproducing stable lower-index tie-breaking exactly);
  chunk ids enumerated by max8-extracting selM * (p*C + c) (block-relative
  chunk index -- doubles as the gather/scatter offset, no per-partition
  scalar math); indirect-DMA gather those 32 chunks/row from the block's
  slice of x (one [P,1]-offset DMA per chunk ordinal); the same
  max8+match_replace trick picks the top-32 elements of the gathered data;
  masked chunks (Gf - zapped) are indirect-DMA scattered into the pre-zeroed
  xz output, so only 1/32 of xz is ever written.

Schedule shaping: block b's stream emission interleaves the PREVIOUS
  block's G-stage DVE ops and scatters, so the select->gather->mask->scatter
  pipeline drains concurrently with streaming instead of trailing the whole
  kernel (the old all-stream-first priority scheme left a ~180us DMA-idle
  tail).  Priority tiers: loads/relu/stores > reduces > M-stage+gathers >
  G-stage > scatters.  Scatter->scatter WAW deps (full-tensor APs, provably
  disjoint chunks) are stripped.
"""

import os
import sys

if "/opt/trn_rl_repo" not in sys.path:
    sys.path.insert(0, "/opt/trn_rl_repo")

import numpy as np

import concourse.bass as bass
import concourse.mybir as mybir
from concourse import bacc
from concourse.bass_utils import run_bass_kernel_spmd
from concourse.tile import TileContext

F32 = mybir.dt.float32
I32 = mybir.dt.int32

N_ROWS = 4096
N_COLS = 32768
N_CORES = 8
K = 32           # top-k
P = 128          # rows per block (partitions)

PRIO_STREAM = -3_000_000
PRIO_G = -2_600_000      # G-stage early: it gates the scatter drain
PRIO_REDUCE = -2_500_000
PRIO_SCAT = -2_200_000   # scatters ahead of next block's gathers on Pool
PRIO_M = -2_000_000

LAST_EXEC_TIME_NS = None
LAST_TRACE_DIR = None
_CACHED_NC = None


def _set_prio(h, delta):
    ins = getattr(h, "ins", h)
    if ins.bass_priority is not None:
        ins.bass_priority += delta


def _build(R: int, D: int, sub: int = 4096, g_bufs: int = 2, x_bufs: int = 8,
           m_bufs: int = 2, s_bufs: int = 4, prio: bool = True, cl: int = 32,
           mode: str = "ind", scratch: int = 32768, gw: int = 1):
    assert mode == "ind"
    assert K % (8 * gw) == 0 or gw in (8, 16, 32)
    C = D // cl
    n_blocks = R // P
    n_sub = D // sub
    sub_chunks = sub // cl

    nc = bacc.Bacc("TRN2", target_bir_lowering=False, debug=False,
                   dynamic_dma_scratch_size=scratch)
    x = nc.declare_dram_parameter("x", [R, D], F32, isOutput=False)
    y = nc.declare_dram_parameter("y", [R, D], F32, isOutput=True)
    xz = nc.declare_dram_parameter("xz", [R, D], F32, isOutput=True)

    x_chunks = x[:].rearrange("r (c l) -> (r c) l", l=cl)
    xz_chunks = xz[:].rearrange("r (c l) -> (r c) l", l=cl)

    with TileContext(nc) as tc:
        with (
            tc.tile_pool(name="consts", bufs=1) as const_pool,
            tc.tile_pool(name="xstream", bufs=x_bufs) as x_pool,
            tc.tile_pool(name="mstage", bufs=m_bufs) as m_pool,
            tc.tile_pool(name="gstage", bufs=g_bufs) as g_pool,
            tc.tile_pool(name="small", bufs=s_bufs) as s_pool,
        ):
            # iota2[p, c] = P*C - (p*C + c): reversed block-relative chunk
            # index (never 0, so the selM mask zeros are unambiguous).  max8
            # extraction of the reversed ids yields ASCENDING chunk ids, so
            # gathered chunks land in ascending-index order and match_replace
            # reproduces top_k's stable lower-index tie-breaking exactly.
            iota_i = const_pool.tile([P, C], I32, tag="iota_i")
            nc.gpsimd.iota(iota_i[:], pattern=[[-1, C]], base=P * C,
                           channel_multiplier=-C)
            iota2 = const_pool.tile([P, C], F32, tag="iota2")
            nc.vector.tensor_copy(iota2[:], iota_i[:])

            # All scatters write the same full-tensor xz AP, so Tile chains
            # them with WAW completion semaphores.  The chunk destinations
            # are provably disjoint; strip scatter->scatter deps.
            scatter_names = set()

            def make_pending(sb, s_sel, G):
                """Build thunk list for block sb's G-stage + scatters."""
                Gf = G[:].rearrange("p k l -> p (k l)")
                Gw = g_pool.tile([P, K * cl], F32, tag="Gw")
                gx8 = s_pool.tile([P, 8], F32, tag="gx8")
                thunks = []

                def g_round(src):
                    def _t():
                        h1 = nc.vector.max(gx8[:], src)
                        h2 = nc.vector.match_replace(out=Gw[:], in_to_replace=gx8[:],
                                                     in_values=src, imm_value=0.0)
                        if prio:
                            _set_prio(h1, PRIO_G)
                            _set_prio(h2, PRIO_G)
                    return _t

                src = Gf
                for _ in range(K // 8):
                    thunks.append(g_round(src))
                    src = Gw[:]

                def sub_t():
                    h = nc.vector.tensor_tensor(out=Gw[:], in0=Gf, in1=Gw[:],
                                                op=mybir.AluOpType.subtract)
                    if prio:
                        _set_prio(h, PRIO_G)
                thunks.append(sub_t)

                Gw3 = Gw[:].rearrange("p (k l) -> p k l", l=cl)

                def scat_group(k0):
                    def _t():
                        for k in range(k0, k0 + 8, gw):
                            if gw == 1:
                                in_ap = Gw3[:, k, :]
                            else:
                                in_ap = Gw3[:, k:k + gw, :]
                            h = nc.gpsimd.indirect_dma_start(
                                out=xz_chunks,
                                out_offset=bass.IndirectOffsetOnAxis(
                                    ap=s_sel[:, k:k + gw], axis=0),
                                in_=in_ap,
                                in_offset=None,
                            )
                            if prio:
                                _set_prio(h, PRIO_SCAT)
                            ins = getattr(h, "ins", h)
                            for dep in list(ins.sync_dependency_names()):
                                if dep in scatter_names:
                                    ins.try_remove_dependency(dep)
                            scatter_names.add(ins.name)
                    return _t

                for k0 in range(0, K, 8):
                    thunks.append(scat_group(k0))
                return thunks

            # Stores lag loads by LAG sub-tile slots (a global queue across
            # blocks) and alternate between the SP and ACT HWDGE rings, as do
            # the loads: two rings keep two transfers in flight (measured
            # ~430 GB/s aggregate vs ~320 for one ring).  Loads are emitted
            # (and ring-issued) before the block's relus so the block's last
            # load -- which gates the M-stage and the whole select pipeline --
            # is never queued behind stores or relu completions.
            LAG = 4
            store_q = []  # (r0, c0, xt, ring) awaiting store emission

            def emit_store():
                r0s, c0s, xts, eng = store_q.pop(0)
                h = eng.dma_start(out=y[r0s:r0s + P, c0s:c0s + sub], in_=xts[:])
                if prio:
                    _set_prio(h, PRIO_STREAM)

            pending = []  # thunks from the previous block
            for b in range(n_blocks):
                r0 = b * P
                M = m_pool.tile([P, C], F32, tag="M")
                n_pend = len(pending)
                # ring-issue all 8 loads first (alternating rings)
                xts = []
                for s in range(n_sub):
                    c0 = s * sub
                    xt = x_pool.tile([P, sub], F32, tag="xt")
                    eng = nc.sync if s % 2 == 0 else nc.scalar
                    h = eng.dma_start(out=xt[:], in_=x[r0:r0 + P, c0:c0 + sub])
                    if prio:
                        _set_prio(h, PRIO_STREAM)
                    xts.append(xt)
                # then per sub-tile: chunk-max reduce on RAW x (selection is
                # relu-invariant), relu in place afterwards, store lagged.
                for s in range(n_sub):
                    c0 = s * sub
                    xt = xts[s]
                    hr = nc.vector.tensor_reduce(
                        out=M[:, s * sub_chunks:(s + 1) * sub_chunks],
                        in_=xt[:].rearrange("p (c l) -> p c l", l=cl),
                        axis=mybir.AxisListType.X,
                        op=mybir.AluOpType.max,
                    )
                    ha = nc.scalar.activation(xt[:], xt[:],
                                              mybir.ActivationFunctionType.Relu)
                    if prio:
                        _set_prio(hr, PRIO_REDUCE)
                        _set_prio(ha, PRIO_STREAM)
                    store_q.append((r0, c0, xt, nc.sync if s % 2 == 0 else nc.scalar))
                    if len(store_q) > LAG:
                        emit_store()
                    # emit this slot's share of the pending thunks
                    lo = n_pend * s // n_sub
                    hi = n_pend * (s + 1) // n_sub
                    for t in pending[lo:hi]:
                        t()
                pending = []

                # top-32 chunks by chunk max (exact stable tie-breaking)
                m_h = []
                Mw = m_pool.tile([P, C], F32, tag="Mw")
                mx8 = s_pool.tile([P, 8], F32, tag="mx8")
                src = M
                for _ in range(K // 8):
                    m_h.append(nc.vector.max(mx8[:], src[:]))
                    m_h.append(nc.vector.match_replace(out=Mw[:], in_to_replace=mx8[:],
                                                       in_values=src[:], imm_value=-1.0))
                    src = Mw
                selM = m_pool.tile([P, C], F32, tag="selM")
                m_h.append(nc.vector.tensor_tensor(out=selM[:], in0=Mw[:], in1=M[:],
                                                   op=mybir.AluOpType.not_equal))
                ids = m_pool.tile([P, C], F32, tag="ids")
                m_h.append(nc.vector.tensor_tensor(out=ids[:], in0=selM[:], in1=iota2[:],
                                                   op=mybir.AluOpType.mult))

                # id extraction rounds interleaved with gather issue
                idsel = s_pool.tile([P, K], F32, tag="idsel")
                sel = s_pool.tile([P, K], I32, tag="sel")
                G = g_pool.tile([P, K, cl], F32, tag="G")
                g_h = []
                for r in range(K // 8):
                    sl = slice(r * 8, (r + 1) * 8)
                    m_h.append(nc.vector.max(idsel[:, sl], ids[:]))
                    if r < K // 8 - 1:
                        m_h.append(nc.vector.match_replace(
                            out=ids[:], in_to_replace=idsel[:, sl],
                            in_values=ids[:], imm_value=0.0))
                    # indirect APs need offset 0, so offsets are global chunk
                    # ids: (b+1)*P*C - extracted_reversed_id (immediates only,
                    # avoiding the slow per-partition scalar-pointer op).
                    m_h.append(nc.vector.tensor_scalar(
                        idsel[:, sl], idsel[:, sl], -1.0, float((b + 1) * P * C),
                        op0=mybir.AluOpType.mult, op1=mybir.AluOpType.add))
                    m_h.append(nc.vector.tensor_copy(sel[:, sl], idsel[:, sl]))
                    for k in range(r * 8, (r + 1) * 8, gw):
                        g_h.append(nc.gpsimd.indirect_dma_start(
                            out=G[:, k, :] if gw == 1 else G[:, k:k + gw, :],
                            out_offset=None,
                            in_=x_chunks,
                            in_offset=bass.IndirectOffsetOnAxis(ap=sel[:, k:k + gw], axis=0),
                        ))
                if prio:
                    for hh in m_h:
                        _set_prio(hh, PRIO_M)
                    for hh in g_h:
                        _set_prio(hh, PRIO_M)
                pending = make_pending(b, sel, G)

            while store_q:
                emit_store()
            for t in pending:
                t()
    nc.finalize()
    return nc


def kernel(x: np.ndarray):
    global LAST_EXEC_TIME_NS, LAST_TRACE_DIR, _CACHED_NC
    x = np.ascontiguousarray(np.asarray(x, dtype=np.float32))
    assert x.shape == (N_ROWS, N_COLS), x.shape
    Rs = N_ROWS // N_CORES

    if _CACHED_NC is None:
        _CACHED_NC = _build(Rs, N_COLS)
    nc = _CACHED_NC

    in_maps = [{"x": x[i * Rs:(i + 1) * Rs]} for i in range(N_CORES)]
    tmpdir = None
    if os.environ.get("BASS_TRACE"):
        import tempfile
        tmpdir = tempfile.mkdtemp(prefix="topk_trace_")
        LAST_TRACE_DIR = tmpdir
    res = run_bass_kernel_spmd(nc, in_maps, core_ids=list(range(N_CORES)),
                               tmpdir=tmpdir)
    LAST_EXEC_TIME_NS = res.exec_time_ns

    y = np.concatenate([np.asarray(res.results[i]["y"]).reshape(Rs, N_COLS)
                        for i in range(N_CORES)], axis=0)
    xz = np.concatenate([np.asarray(res.results[i]["xz"]).reshape(Rs, N_COLS)
                         for i in range(N_CORES)], axis=0)
    return y, xz
